# revision 16
# baseline (speedup 1.0000x reference)
"""Banded (sliding-window) multi-head attention for Trainium2, 8 NeuronCores.

Reference computation (fp32):
    q = query @ Wq + bq ; k = key @ Wk + bk ; v = value @ Wv + bv   (per-head split)
    scores = q k^T / sqrt(U), masked to |i-j| <= 128, softmax, out = attn @ v

Sharding: 8 cores = 2 batches x 4 sequence chunks of 512 query rows.
Each core gets its query chunk (transposed), a 768-row padded k/v halo chunk
(transposed), all weights, and a precomputed additive corner-mask pack.

Per-core kernel (SPMD, identical program, different data).

  - q,k projected in fp8(e4m3) DoubleRow mode (0.5 PE cycles/col): inputs and
    16x-scaled weights are pre-interleaved host-side as [128p, chunk, 2, cols]
    with contraction element d = 256*chunk + 2*p + i. Two matmuls per psum
    instead of four, at half the per-column cost. The 16x weight scale keeps
    W (~N(0,0.02)) out of e4m3's subnormal range; it is undone by folding
    1/256 into the exp scale (exp(s'/2048) == exp(s/8)).
  - v projected in bf16 into natural [seq, head*unit] layout with a
    ones-column per head appended so P@V also yields the softmax denom.
  - scoresT[c, r] = k_h^T q_h per kv-tile c (bf16), over only the in-band
    r-window. Corner out-of-band ranges get an additive -57600 mask folded
    into the same PSUM accumulation group by ONE fp8 DoubleRow matmul per
    (head-pair, kv-tile): stationary is a 240*I DoubleRow identity, moving is
    the 0/-240 mask duplicated per head, out is the strided [128, 2, wl] view
    covering both heads' psum regions. Half the cost and half the instruction
    count of the old per-head bf16 mask matmuls.
  - P = exp(scoresT / 2048) on ACT (no max subtraction: |s'| small).
  - out[r, u] = P^T @ v_aug on PE; denominators come out in column U.
  - out *= 1/denom on DVE; one merged [128, 512] DMA per row-tile at the end.

DMA strategy: one (or two, for the pipeline-gating wq/qt) merged descriptor
per DRAM tensor — per-DMA overheads (SEQ issue + DGE + sem-prop) dominate
transfer time at these sizes. q-path tensors ride the sync queue (HWDGE),
k/v-path tensors the gpsimd queue (SWDGE) so descriptor generation runs in
parallel. Input tiles are double-buffered so loop iterations pipeline.
"""

import sys

sys.path.insert(0, "/opt/trn_rl_repo")

import numpy as np
from contextlib import ExitStack

import concourse.bass as bass  # noqa: F401
import concourse.tile as tile
from concourse import bacc, mybir
from concourse.bass_utils import run_bass_kernel_spmd

B, S, D = 2, 2048, 512
H, U = 8, 64
LEFT, RIGHT = 128, 128
NCORES = 8
SC = S // (NCORES // B)  # 512 query rows per core
KC = SC + LEFT + RIGHT  # 768 k/v rows per core (halo)
NJ = KC // 128  # 6 kv column tiles
NT = SC // 128  # 4 query row tiles
KD = D // 128  # 4 contraction tiles (bf16 path: v)
DRC = D // 256  # 2 DoubleRow contraction chunks (fp8 path: q, k)
MH = D // 128  # 4 head-pair tiles ([hu] dim)
# exact in-band r-window (start, len) per kv tile j
WIN = [(0, 128), (0, 256), (0, 384), (128, 384), (256, 256), (384, 128)]

# fp8 q/k path: weights scaled by 16 (out of e4m3 subnormals); scores come
# out 256x large; exp scale folds the 1/256 and the 1/sqrt(U)=1/8 together.
WSCALE = 16.0
EXP_SCALE = 1.0 / (8.0 * WSCALE * WSCALE)
MASK_ID = 240.0  # DoubleRow identity magnitude (e4m3 max normal)
MASK_NEG = -240.0  # mask magnitude; product -57600 -> exp(-28.1) ~ 6e-13

# maskpack (fp8, 128 partitions; rows 64+ are zero padding so every DR
# matmul uses the same (128,128) PE tile config — mixing 64- and 128-row DR
# stationaries in one program wedges the NEFF): per window slot s the
# additive 0/-240 mask in DoubleRow moving layout [128p, 2i, wl] (c = 2p+i
# for p<64), then the 240*I DoubleRow identity [128p, 2i, 128c].
_WSLOT = [0, 1, 2, 2, 2, 3]  # window slot per j
_SLOT_WL = [128, 256, 384, 128]
_SLOT_OFF = [0]
for _wl in _SLOT_WL:
    _SLOT_OFF.append(_SLOT_OFF[-1] + 2 * _wl)
_ID_OFF = _SLOT_OFF[-1]  # 1792
MP_COLS = _ID_OFF + 2 * 128  # 2048

F32 = mybir.dt.float32
BF16 = mybir.dt.bfloat16
F8 = mybir.dt.float8e4
DR = mybir.MatmulPerfMode.DoubleRow
AF = mybir.ActivationFunctionType

_DIAG = "full"   # "full" | "dma" (loads only) | "compute" (tiny loads)
_HINTS = False   # branch-prefetch hints on the timing loop
_UNROLL = 8      # loop bodies per For_i trip: amortizes the per-trip
                 # drain/sem-reset barrier and lets adjacent bodies pipeline
_BODIES = 1      # bodies in the no-loop (correctness/sim) program


def _load_consts(tc: "tile.TileContext", sb, io):
    """Weights and maskpack are loop-invariant: load them once, outside the
    body, single-buffered. Saves ~1.25 MB of DMA + 5 descriptor issues per
    iteration in the steady-state loop."""
    nc = tc.nc
    qT, kT, vT, Wq, Wk, Wv, maskpack, out = io

    def cload(dram, n, width, tag, eng, dt=BF16, rows128=False):
        t = sb.tile([128, n * width], dt, tag=tag, name=tag, bufs=1)
        t3 = t.rearrange("p (n s) -> p n s", n=n)
        if rows128:
            r = dram.rearrange("p (n s) -> p n s", n=n)
        else:
            r = dram.rearrange("(n p) s -> p n s", p=128)
        eng.dma_start(t3[:], r)
        return t3

    wq = cload(Wq, DRC, 2 * D, "wq", nc.sync, dt=F8, rows128=True)
    wk = cload(Wk, DRC, 2 * D, "wk", nc.gpsimd, dt=F8, rows128=True)
    wv = cload(Wv, KD, D, "wv", nc.gpsimd)
    mp_sb = sb.tile([128, MP_COLS], F8, tag="mp", name="mp_sb", bufs=1)
    nc.sync.dma_start(mp_sb[:], maskpack[:])
    return wq, wk, wv, mp_sb


def _emit(ctx: ExitStack, tc: "tile.TileContext", io, loop_k=None):
    sb = ctx.enter_context(tc.tile_pool(name="sb", bufs=1))
    sbr = ctx.enter_context(tc.tile_pool(name="sbr", bufs=1))
    psum = ctx.enter_context(tc.tile_pool(name="psum", bufs=1, space="PSUM"))
    consts = _load_consts(tc, sb, io)
    if loop_k is not None:
        hints = ()
        if _HINTS:
            hints = (
                mybir.EngineType.PE,
                mybir.EngineType.Activation,
                mybir.EngineType.DVE,
                mybir.EngineType.SP,
                mybir.EngineType.Pool,
            )
        n_loop, n_pre = divmod(loop_k, _UNROLL)
        if n_loop == 0:
            n_pre, n_loop = 0, 0
            for _ in range(loop_k):
                _emit_body(tc, io, sb, sbr, psum, consts)
        else:
            for _ in range(n_pre):
                _emit_body(tc, io, sb, sbr, psum, consts)
            with tc.For_i(0, n_loop, 1, hint_engines=hints):
                for _ in range(_UNROLL):
                    _emit_body(tc, io, sb, sbr, psum, consts)
    else:
        for _ in range(_BODIES):
            _emit_body(tc, io, sb, sbr, psum, consts)


def _emit_body(tc: "tile.TileContext", io, sb, sbr, psum, consts):
    nc = tc.nc
    qT, kT, vT, Wq, Wk, Wv, maskpack, out = io
    wq, wk, wv, mp_sb = consts

    # merged input loads: one tile holding all chunks of a DRAM tensor,
    # filled by one (or two) multi-dim DMA descriptor sets. rows128=True
    # means the DRAM tensor is already laid out [128, n*width] (the fp8
    # DoubleRow packs); otherwise it is [n*128, width] row-blocked.
    def mload(dram, n, width, tag, eng, dt=BF16, split_first=False, rows128=False):
        t = sb.tile([128, n * width], dt, tag=tag, name=tag, bufs=2)
        t3 = t.rearrange("p (n s) -> p n s", n=n)
        if rows128:
            r = dram.rearrange("p (n s) -> p n s", n=n)
        else:
            r = dram.rearrange("(n p) s -> p n s", p=128)
        if _DIAG == "compute":
            eng.dma_start(t3[0:1, 0:1, :], r[0:1, 0:1, :])
        elif split_first:
            eng.dma_start(t3[:, 0:1, :], r[:, 0:1, :])
            eng.dma_start(t3[:, 1:n, :], r[:, 1:n, :])
        else:
            eng.dma_start(t3[:], r)
        return t3

    # q-path on sync (HWDGE), k/v-path on gpsimd (SWDGE): the two descriptor
    # generators run in parallel. wq/qt chunk 0 are split out so the first
    # projection matmul can start as soon as the leading bytes have landed.
    # fp8 DoubleRow tensors are [128, chunk, 2*cols]; slice per chunk then
    # rearrange to the [128, 2, cols] AP the DR matmul wants.
    qt = mload(qT, DRC, 2 * SC, "qt", nc.sync, dt=F8, split_first=True,
               rows128=True)
    kt = mload(kT, DRC, 2 * KC, "kt", nc.gpsimd, dt=F8, rows128=True)
    vt = mload(vT, KD, KC, "vt", nc.gpsimd)

    def dr2(t3, c):
        # [128, c-chunk, 2*w] -> [128, 2, w] DoubleRow pair view of chunk c
        w = t3.shape[2] // 2
        return t3[:, c].rearrange("p (i w) -> p i w", i=2)

    id_dr = mp_sb[:, _ID_OFF : _ID_OFF + 256].rearrange("p (i c) -> p i c", i=2)

    def mp_dr(j, wl):
        # slot layout is [p, 2i, slot_wl]; shorter windows (j=4 reusing the
        # 384 slot) crop r but must keep the slot's i-stride.
        s = _WSLOT[j]
        off, swl = _SLOT_OFF[s], _SLOT_WL[s]
        v = mp_sb[:, off : off + 2 * swl].rearrange("p (i r) -> p i r", i=2)
        return v[:, :, 0:wl]

    q_sb, k_sb = [], []

    def proj_qk(m):
        ps = psum.tile([128, SC], F32, tag="ps", bufs=2, name=f"qp{m}")
        for c in range(DRC):
            nc.tensor.matmul(
                ps[:], dr2(wq, c)[:, :, m * 128 : (m + 1) * 128], dr2(qt, c),
                start=(c == 0), stop=(c == DRC - 1), perf_mode=DR,
            )
        qsb = sb.tile([128, SC], BF16, tag=f"q{m}", name=f"q{m}", bufs=2)
        nc.vector.tensor_copy(qsb[:], ps[:])
        q_sb.append(qsb)

        ksb = sb.tile([128, KC], BF16, tag=f"k{m}", name=f"k{m}", bufs=2)
        for c0, cl in ((0, 512), (512, 256)):
            ps = psum.tile([128, cl], F32, tag="ps", bufs=2, name=f"kp{m}_{c0}")
            for c in range(DRC):
                nc.tensor.matmul(
                    ps[:], dr2(wk, c)[:, :, m * 128 : (m + 1) * 128],
                    dr2(kt, c)[:, :, c0 : c0 + cl],
                    start=(c == 0), stop=(c == DRC - 1), perf_mode=DR,
                )
            nc.vector.tensor_copy(ksb[:, c0 : c0 + cl], ps[:])
        k_sb.append(ksb)

    # v in natural [seq, hu] layout, 65 cols/head (65th = 1.0 for the denom).
    # bv is spec'd all-zeros, so no bias term is added.
    v_sb = []

    def proj_v(m):
        vs = sbr.tile([128, H * (U + 1)], BF16, tag=f"v{m}", name=f"v{m}", bufs=2)
        vs3 = vs.rearrange("p (h u) -> p h u", h=H)
        nc.gpsimd.memset(vs3[:, :, U : U + 1], 1.0)
        ps = psum.tile([128, D], F32, tag="ps", bufs=2, name=f"vp{m}")
        for k in range(KD):
            nc.tensor.matmul(
                ps[:], vt[:, k, m * 128 : (m + 1) * 128], wv[:, k],
                start=(k == 0), stop=(k == KD - 1),
            )
        nc.vector.tensor_copy(vs3[:, :, 0:U], ps.rearrange("p (h u) -> p h u", h=H))
        v_sb.append(vs)

    out_sb = [
        sb.tile([128, D], BF16, tag=f"o{t}", name=f"o{t}", bufs=2) for t in range(NT)
    ]
    if _DIAG in ("nopv", "scoresonly", "projonly"):
        for t in range(NT):
            nc.gpsimd.memset(out_sb[t][:], 0.0)
    pts = {}

    def scores_exp_pair(pair, j, pv_thunks=()):
        # Both heads' score windows in one 2-bank PSUM tile (offsets 0/512);
        # one fp8-DR mask matmul (half cost) + one scores matmul per head.
        # A PV thunk is dripped after each long-stream matmul so its weight
        # load overlaps that stream.
        m = pair[0] // 2
        w0, wl = WIN[j]
        it = iter(pv_thunks)
        sp = psum.tile([128, 1024], F32, tag="sc2", bufs=2, name=f"s{m}_{j}")
        sp3 = sp.rearrange("p (h c) -> p h c", h=2)
        if _DIAG != "nomask":
            for hh in (0, 1):
                nc.tensor.matmul(
                    sp[:, hh * 512 : hh * 512 + wl], id_dr, mp_dr(j, wl),
                    start=True, stop=False, perf_mode=DR, skip_group_check=True,
                )
                th = next(it, None)
                if th:
                    th()
        for hh in (0, 1):
            dh = hh * 64
            nc.tensor.matmul(
                sp[:, hh * 512 : hh * 512 + wl],
                k_sb[m][dh : dh + 64, j * 128 : (j + 1) * 128],
                q_sb[m][dh : dh + 64, w0 : w0 + wl],
                start=(_DIAG == "nomask"), stop=True, skip_group_check=True,
            )
            th = next(it, None)
            if th:
                th()
        for th in it:
            th()
        if _DIAG == "scoresonly":
            return
        pt = sbr.tile([128, 2, 384], BF16, tag="pt", bufs=12, name=f"pt{m}_{j}")
        nc.scalar.activation(pt[:, :, 0:wl], sp3[:, :, 0:wl], AF.Exp, scale=EXP_SCALE)
        for hh in (0, 1):
            pts[(pair[hh], j)] = pt[:, hh, :]

    def pv_matmuls(pair, t):
        """PV for (pair, t) as a list of single-matmul thunks plus a finish
        closure (DVE recip + scale). The thunks are dripped one-per-matmul
        into the scores emission so each PV weight load (128 rows, stream
        only 65) hides under a long mask/scores stream instead of the
        previous PV matmul's 65-col stream."""
        if _DIAG in ("nopv", "scoresonly", "projonly"):
            return [], None
        # both heads of the pair share one PSUM bank: [128, 2*65]
        op = psum.tile([128, 2 * (U + 1)], F32, tag="pvp", bufs=2,
                       name=f"ov{pair[0]}_{t}")
        thunks = []
        for hh, h in enumerate(pair):
            for i, j in enumerate((t, t + 1, t + 2)):
                w0 = WIN[j][0]
                def mk(hh=hh, h=h, i=i, j=j, w0=w0):
                    nc.tensor.matmul(
                        op[:, hh * (U + 1) : (hh + 1) * (U + 1)],
                        pts[(h, j)][:, t * 128 - w0 : t * 128 - w0 + 128],
                        v_sb[j][:, h * (U + 1) : (h + 1) * (U + 1)],
                        start=(i == 0), stop=(i == 2),
                    )
                thunks.append(mk)

        def finish():
            op3 = op.rearrange("p (h u) -> p h u", h=2)
            rec = sbr.tile([128, 2], F32, tag="rec", bufs=8,
                           name=f"rec{pair[0]}_{t}")
            nc.vector.reciprocal(rec[:], op3[:, :, U : U + 1])
            m = pair[0] // 2
            ot = out_sb[t][:, m * 128 : (m + 1) * 128].rearrange(
                "p (h u) -> p h u", h=2
            )
            nc.vector.tensor_tensor(
                ot, op3[:, :, 0:U],
                rec[:].rearrange("p (h o) -> p h o", o=1).to_broadcast((128, 2, U)),
                op=mybir.AluOpType.mult,
            )

        return thunks, finish

    def pv_pair(pair, t):
        thunks, finish = pv_matmuls(pair, t)
        for th in thunks:
            th()
        if finish:
            finish()

    def out_dma(t, c0=0, c1=D):
        nc.sync.dma_start(
            out[t * 128 : (t + 1) * 128, c0:c1], out_sb[t][:, c0:c1]
        )

    if _DIAG in ("dma", "dma4"):
        zt = sb.tile([128, D], BF16, tag="o0", name="zt")
        nc.vector.memset(zt[:], 0.0)
        for t in range(NT):
            nc.sync.dma_start(out[t * 128 : (t + 1) * 128, :], zt[:])
        return

    # ---- schedule: head-pair m only needs projection m-tile m. PV for
    # tile t fires one j-step after its last window's exp (at j=t+3, and
    # each pair's final t=3 inside the next pair's first slot) so the
    # in-order PE queue doesn't stall on ACT's exp latency. PV psums live
    # in their own pool tag so the deferred PV can't alias a projection
    # psum mid-rotation. ----
    proj_qk(0)
    for m in range(3):
        proj_v(m)
    prev_pair = None
    for m in range(MH):
        pair = (2 * m, 2 * m + 1)
        for j in range(NJ):
            if _DIAG != "projonly":
                scores_exp_pair(pair, j)
            if j == 0 and prev_pair is not None:
                pv_pair(prev_pair, NT - 1)
                if m == MH - 1:
                    # tile 3: cols 0-383 go out here; only the final
                    # 128-col block rides the kernel tail
                    out_dma(NT - 1, 0, 384)
            if j >= 3:
                pv_pair(pair, j - 3)
                if m == MH - 1:
                    out_dma(j - 3)
            if m == 0 and j == 0:
                for vm in range(3, NJ):
                    proj_v(vm)
            if j == 1 and m + 1 < MH:
                proj_qk(m + 1)
        prev_pair = pair
    pv_pair(prev_pair, NT - 1)
    out_dma(NT - 1, 384, D)


_PROGRAMS = {}


def build_program(loop_k=None):
    key = (loop_k, _DIAG, _HINTS, _UNROLL, _BODIES)
    if key in _PROGRAMS:
        return _PROGRAMS[key]
    nc = bacc.Bacc("TRN2", target_bir_lowering=False, debug=False, num_devices=NCORES)
    io = (
        nc.dram_tensor("qT", [128, DRC * 2 * SC], F8, kind="ExternalInput").ap(),
        nc.dram_tensor("kT", [128, DRC * 2 * KC], F8, kind="ExternalInput").ap(),
        nc.dram_tensor("vT", [D, KC], BF16, kind="ExternalInput").ap(),
        nc.dram_tensor("Wq", [128, DRC * 2 * D], F8, kind="ExternalInput").ap(),
        nc.dram_tensor("Wk", [128, DRC * 2 * D], F8, kind="ExternalInput").ap(),
        nc.dram_tensor("Wv", [D, D], BF16, kind="ExternalInput").ap(),
        nc.dram_tensor("maskpack", [128, MP_COLS], F8, kind="ExternalInput").ap(),
        nc.dram_tensor("out", [SC, D], BF16, kind="ExternalOutput").ap(),
    )
    with tile.TileContext(nc) as tc:
        with ExitStack() as ctx:
            _emit(ctx, tc, io, loop_k=loop_k)
    nc.compile()
    _PROGRAMS[key] = nc
    return nc


def _band_win(j, q0, k0):
    """[128, wl] additive mask (0 / MASK_NEG) for kv tile j's full window."""
    w0, wl = WIN[j]
    c_glob = k0 + j * 128 + np.arange(128)
    r_glob = q0 + w0 + np.arange(wl)
    valid = (
        (np.abs(r_glob[None, :] - c_glob[:, None]) <= LEFT)
        & (c_glob[:, None] >= 0)
        & (c_glob[:, None] < S)
    )
    return np.where(valid, 0.0, MASK_NEG)


def _dr_pack(x):
    """[512, cols] -> [128, chunk, 2*cols] fp8 DoubleRow interleave with
    contraction element d = 256*chunk + 2*p + i."""
    import ml_dtypes

    cols = x.shape[1]
    x4 = x.reshape(DRC, 128, 2, cols)  # (chunk, p, i, r)
    x4 = x4.transpose(1, 0, 2, 3).reshape(128, DRC * 2 * cols)
    return np.ascontiguousarray(x4).astype(ml_dtypes.float8_e4m3)


def _core_inputs(query, key, value, Wq, Wk, Wv, bq, bk, bv, b, t):
    import ml_dtypes

    bf = ml_dtypes.bfloat16
    q0 = t * SC
    k0 = q0 - LEFT
    qT = _dr_pack(np.ascontiguousarray(query[b, q0 : q0 + SC, :].T))
    kpad = np.zeros((KC, D), np.float32)
    vpad = np.zeros((KC, D), np.float32)
    lo, hi = max(0, k0), min(S, q0 + SC + RIGHT)
    kpad[lo - k0 : hi - k0] = key[b, lo:hi, :]
    vpad[lo - k0 : hi - k0] = value[b, lo:hi, :]
    kT = _dr_pack(np.ascontiguousarray(kpad.T))
    vT = np.ascontiguousarray(vpad.T).astype(bf)

    maskpack = np.zeros((128, MP_COLS), np.float32)
    for s, j in ((0, 0), (1, 1), (2, 2), (3, 5)):
        wl = WIN[j][1]
        band = _band_win(j, q0, k0)  # [128 c, wl]
        mp3 = band.reshape(64, 2, wl)  # (p, i, r): c = 2p+i
        maskpack[:64, _SLOT_OFF[s] : _SLOT_OFF[s] + 2 * wl] = mp3.reshape(
            64, 2 * wl)
    idblk = maskpack[:64, _ID_OFF : _ID_OFF + 256].reshape(64, 2, 128)
    for p in range(64):
        for i in range(2):
            idblk[p, i, 2 * p + i] = MASK_ID
    # j=3/j=4 share slot 2's pattern (their leading wl cols) — verify:
    for j in (3, 4):
        wl = WIN[j][1]
        blk = maskpack[:64, _SLOT_OFF[2] : _SLOT_OFF[2] + 2 * WIN[2][1]].reshape(
            64, 2, WIN[2][1]
        )
        assert (blk[:, :, :wl] == _band_win(j, q0, k0).reshape(64, 2, wl)).all()

    return {
        "qT": qT, "kT": kT, "vT": vT,
        "Wq": _dr_pack(Wq * WSCALE), "Wk": _dr_pack(Wk * WSCALE),
        "Wv": Wv.astype(bf),
        "maskpack": maskpack.astype(ml_dtypes.float8_e4m3),
    }


def make_in_maps(inputs):
    f = {k: np.asarray(v, dtype=np.float32) for k, v in inputs.items()}
    in_maps = []
    for core in range(NCORES):
        b, t = core // NT, core % NT
        in_maps.append(
            _core_inputs(
                f["query"], f["key"], f["value"],
                f["Wq"], f["Wk"], f["Wv"], f["bq"], f["bk"], f["bv"], b, t,
            )
        )
    return in_maps


def run(inputs, trace=False):
    """Returns (output, BassKernelResults)."""
    nc = build_program()
    in_maps = make_in_maps(inputs)
    res = run_bass_kernel_spmd(nc, in_maps, list(range(NCORES)), trace=trace)
    out = np.empty((B, S, D), np.float32)
    for core in range(NCORES):
        b, t = core // NT, core % NT
        out[b, t * SC : (t + 1) * SC, :] = res.results[core]["out"].astype(
            np.float32
        )
    return out, res


def kernel(**inputs):
    out, _ = run(inputs)
    return out
